# revision 41
# baseline (speedup 1.0000x reference)
"""Local-window (banded) multi-head attention on 8 Trainium2 NeuronCores.

Problem: x[L=2048, B=8, D=512], Wqkv[1536, 512], Wout[512, 512], bout[512].
  qkv = x @ Wqkv.T ; per-head banded attention (|i-j| <= 64, window 129);
  out = attn_out @ Wout.T + bout.

Sharding: batch B=8 across the 8 cores (data parallel).

Per-core structure (all matmuls contract over the partition dim):
 - Q/K projection in fp8e4m3 DoubleRow (weights pre-scaled x16 host-side;
   compensated in the exp scale 2^-11). q/k noise is attenuated ~10x through
   the softmax, so fp8 is safe here.
 - V projection in bf16, written directly into 64-row-shifted tiles (17
   half-overlapping tiles) so the banded PV needs no re-blocking copies.
 - Banded scores computed transposed (scoresT[m, l]) per 128-query chunk
   over a 256-key window: 2 m-tiles, bf16. Band masking is done on the PE:
   an fp8e5 DoubleRow matmul adds a constant -57344 upper/lower-triangular
   matrix into the score PSUM (exp then underflows to exactly 0).
 - exp on the scalar engine -> P in bf16 (P in fp8 would cost ~2e-2 rel
   error; bf16 keeps it at ~1e-2 total).
 - PV in bf16 with head pairs STACKED on partitions (out offsets 0/64),
   so normalization is elementwise. Denominators come from fp8 DoubleRow
   ones-matmuls against an fp8 copy of P (sum averaging kills the fp8
   noise), duplicated across 64 rows so the divide needs no broadcast.
 - Output projection in bf16; bias-add + store as bf16.
"""

import sys

import numpy as np
import ml_dtypes

if "/opt/trn_rl_repo" not in sys.path:
    sys.path.insert(0, "/opt/trn_rl_repo")

L, B, D, H, DH = 2048, 8, 512, 8, 64
NCH = L // 128  # 16 attention chunks
NVT = 17  # shifted V tiles
NEG = -57344.0  # e5m2-exact mask value; exp(2^-11 * -57344) == 0 in bf16
WS = 16.0  # host-side Q/K weight scale (keeps fp8 out of subnormals)

_NC_CACHE = {}


def _build_nc(with_bout=True):
    from concourse import bacc, mybir, tile

    f32 = mybir.dt.float32
    f16 = mybir.dt.float16
    bf16 = mybir.dt.bfloat16
    f8 = mybir.dt.float8e4
    e5 = mybir.dt.float8e5
    Exp = mybir.ActivationFunctionType.Exp
    DR = mybir.MatmulPerfMode.DoubleRow

    import concourse.bass as bass

    nc = bacc.Bacc(None, target_bir_lowering=False)

    x8_d = nc.dram_tensor("x8", [128, 8704], f8, kind="ExternalInput")
    dx8_d = nc.dram_tensor("dx8", [128, 8704], f8, kind="ExternalInput")
    wqk_d = nc.dram_tensor("wqk8", [128, 4096], f8, kind="ExternalInput")
    wv_d = nc.dram_tensor("wv8", [128, 4096], f8, kind="ExternalInput")
    wo_d = nc.dram_tensor("wob", [128, 2048], bf16, kind="ExternalInput")
    one8_d = nc.dram_tensor("one8", [128, 128], f8, kind="ExternalInput")
    ce5_d = nc.dram_tensor("ce5", [128, 3328], e5, kind="ExternalInput")
    bout_d = nc.dram_tensor("bout", [1, 512], bf16, kind="ExternalInput")
    y_d = nc.dram_tensor("y", [L, D], bf16, kind="ExternalOutput")

    def mm(out, lhsT, rhs, start, stop, pm=None):
        nc.tensor.matmul(out, lhsT, rhs, start=start, stop=stop, perf_mode=pm)

    with tile.TileContext(nc) as tc, nc.allow_low_precision(
        reason="fp8/bf16 tiles feed the PE fast paths; accumulation is fp32"
    ):
        with (
            tc.tile_pool(name="pers", bufs=1) as pers,
            tc.tile_pool(name="ps", bufs=1, space="PSUM") as ps,
        ):
            xt8 = pers.tile([128, 8704], f8, name="xt8", tag="xt8")
            dx8 = pers.tile([128, 8704], f8, name="dx8", tag="dx8")
            wqk = pers.tile([128, 4096], f8, name="wqk", tag="wqk")
            wv = pers.tile([128, 4096], f8, name="wv", tag="wv")
            wo = pers.tile([128, 2048], bf16, name="wo", tag="wo")
            one8 = pers.tile([128, 128], f8, name="one8", tag="one8")
            ce5 = pers.tile([128, 3328], e5, name="ce5", tag="ce5")
            boutr = pers.tile([1, 512], bf16, name="boutr", tag="boutr")
            onecol = pers.tile([1, 128], bf16, name="onecol", tag="onecol")
            QT = [pers.tile([128, 2048], bf16, name=f"QT{t}", tag=f"QT{t}")
                  for t in range(4)]
            KT = [pers.tile([128, 2176], bf16, name=f"KT{t}", tag=f"KT{t}")
                  for t in range(4)]
            VA = pers.tile([128, NVT * 512], bf16, name="VA", tag="VA")

            # ---- input DMAs, sliced so phase B unblocks early ----
            # Q weights + x8 for lc=0 first, then K weights, V path, rest.
            nc.sync.dma_start(out=wqk[:, 0:2048], in_=wqk_d[:, 0:2048])
            for kb in range(4):
                c0 = 2176 * kb
                nc.sync.dma_start(
                    out=xt8[:, c0 : c0 + 704], in_=x8_d[:, c0 : c0 + 704]
                )
            nc.sync.dma_start(out=wqk[:, 2048:4096], in_=wqk_d[:, 2048:4096])
            nc.sync.dma_start(out=wv[:], in_=wv_d[:])
            for kb in range(4):
                c0 = 2176 * kb
                nc.sync.dma_start(
                    out=dx8[:, c0 : c0 + 704], in_=dx8_d[:, c0 : c0 + 704]
                )
            nc.sync.dma_start(out=ce5[:], in_=ce5_d[:])
            nc.sync.dma_start(out=one8[:], in_=one8_d[:])
            nc.sync.dma_start(out=boutr[:], in_=bout_d[:])
            nc.vector.memset(onecol[:], 1.0)
            # remaining x slices per lc
            for lc in range(1, 4):
                w = 448 if lc == 3 else 512
                for kb in range(4):
                    c0 = 2176 * kb + 704 + 512 * (lc - 1)
                    nc.sync.dma_start(
                        out=xt8[:, c0 : c0 + w], in_=x8_d[:, c0 : c0 + w]
                    )
                for kb in range(4):
                    c0 = 2176 * kb + 704 + 512 * (lc - 1)
                    nc.sync.dma_start(
                        out=dx8[:, c0 : c0 + w], in_=dx8_d[:, c0 : c0 + w]
                    )
            nc.sync.dma_start(out=wo[:], in_=wo_d[:])
            # KT zero pads (left 64, right 64)
            for t in range(4):
                nc.vector.memset(KT[t][:, 0:64], 0.0)
                nc.vector.memset(KT[t][:, 2112:2176], 0.0)

            ident = ce5[:, 0:256].rearrange("p (i m) -> p i m", i=2)  # [I|Z]

            def utri_rhs(u):
                # [u_r0h0 | u_r1h0 | u_r0h1 | u_r1h1] then 512 zeros
                return ce5[:, 256 + 1024 * u : 256 + 1024 * u + 1024].rearrange(
                    "p (i m) -> p i m", i=2
                )

            onesv = one8[:, 0:128].rearrange("p (i m) -> p i m", i=2)[:, :, 0:64]

            def emit_qk_proj(t, lc, is_k):
                # psum [128ch, 512l] = DR over 2 k-pairs
                wt = t + 4 * is_k
                pj = ps.tile([128, 512], f32, name=f"pj{wt}_{lc}", tag="sc", bufs=2)
                for jj in range(2):
                    lhsT = wqk[:, 512 * wt + 256 * jj : 512 * wt + 256 * jj + 256
                               ].rearrange("p (i m) -> p i m", i=2)
                    rhs = xt8[:, 4352 * jj : 4352 * jj + 4352].rearrange(
                        "p (i l) -> p i l", l=2176
                    )[:, :, 64 + 512 * lc : 64 + 512 * lc + 512]
                    mm(pj[:], lhsT, rhs, start=(jj == 0), stop=(jj == 1), pm=DR)
                if is_k:
                    dest = KT[t][:, 64 + 512 * lc : 64 + 512 * lc + 512]
                    nc.scalar.copy(out=dest, in_=pj[:])
                else:
                    dest = QT[t][:, 512 * lc : 512 * lc + 512]
                    nc.vector.tensor_copy(out=dest, in_=pj[:])

            def emit_v_proj(j):
                # V tile j covers l in [128j-64, 128j+64); fp8 DoubleRow,
                # 3 terms: x8@Wv8 + x8@dWv8 + dx8@Wv8 (residual-compensated)
                vp = ps.tile([128, 512], f32, name=f"vp{j}", tag="sc", bufs=2)
                first = True
                for xa, wa in ((xt8, 0), (xt8, 2048), (dx8, 0)):
                    for jj in range(2):
                        lhsT = xa[:, 4352 * jj : 4352 * jj + 4352].rearrange(
                            "p (i l) -> p i l", l=2176
                        )[:, :, 128 * j : 128 * j + 128]
                        rhs = wv[:, wa + 1024 * jj : wa + 1024 * jj + 1024
                                 ].rearrange("p (i n) -> p i n", n=512)
                        last = (xa is dx8) and jj == 1
                        mm(vp[:], lhsT, rhs, start=first, stop=last, pm=DR)
                        first = False
                # scatter into per-head 64-col blocks: col = 1088h + 64j + e
                dst = VA.rearrange("p (h c) -> p h c", h=H)[
                    :, :, 64 * j : 64 * j + 64
                ]
                src = vp.rearrange("p (h e) -> p h e", e=64)
                nc.vector.tensor_copy(out=dst, in_=src)

            vtiles = [range(0, 5), range(5, 9), range(9, 13), range(13, 17)]

            def emit_scores(c):
                # scores for 2 pairs per [128,1024] psum tile; one exp + one
                # fp8 cast per tile (scalar/gpsimd op count halved)
                ptiles = []
                p8tiles = []
                u = 0
                if c == 0:
                    u = 1
                if c == NCH - 1:
                    u = 2
                for g in range(2):  # pair group: pairs 2g, 2g+1
                    scp = ps.tile([128, 1024], f32, name=f"sc{g}_{c}", tag="sc",
                                  bufs=2)
                    for tt in range(2):
                        t = 2 * g + tt
                        for hh in range(2):
                            p0 = 64 * hh
                            qsl = QT[t][p0 : p0 + 64, 128 * c : 128 * c + 128]
                            for r in range(2):
                                o0 = 512 * tt + 256 * hh + 128 * r
                                out = scp[:, o0 : o0 + 128]
                                ksl = KT[t][p0 : p0 + 64,
                                            128 * c + 128 * r : 128 * c + 128 * r + 128]
                                mm(out, ksl, qsl, start=True, stop=False)
                                mm(out, ident,
                                   utri_rhs(u)[:, :, 128 * (2 * hh + r) :
                                               128 * (2 * hh + r) + 128],
                                   start=False, stop=True, pm=DR)
                    pt = pers.tile([128, 1024], bf16, name=f"pt{g}_{c}", tag="p",
                                   bufs=6)
                    nc.scalar.activation(out=pt[:], in_=scp[:], func=Exp,
                                         scale=0.00048828125)
                    p8 = pers.tile([128, 1024], f8, name=f"p8{g}_{c}", tag="p8",
                                   bufs=6)
                    nc.gpsimd.tensor_copy(out=p8[:], in_=pt[:])
                    ptiles.append(pt)
                    p8tiles.append(p8)
                return c, ptiles, p8tiles

            def emit_pvdn(pend):
                c, ptiles, p8tiles = pend
                op = ps.tile([128, 512], f32, name=f"op{c}", tag="op", bufs=1)
                dn = ps.tile([64, 1024], f32, name=f"dn{c}", tag="dn", bufs=1)
                # denominators first so the DVE recip chain overlaps the PVs
                for t in range(4):
                    p8 = p8tiles[t // 2]
                    b0 = 512 * (t % 2)
                    for hh in range(2):
                        p8v = p8[:, b0 + 256 * hh : b0 + 256 * hh + 256].rearrange(
                            "p (two l) -> p two l", two=2
                        )
                        mm(dn[:, 512 * hh + 128 * t : 512 * hh + 128 * t + 128],
                           onesv, p8v, start=True, stop=True, pm=DR)
                for t in range(4):
                    pt = ptiles[t // 2]
                    b0 = 512 * (t % 2)
                    for hh in range(2):
                        h = 2 * t + hh
                        for r in range(2):
                            vsl = VA[:, 1088 * h + 64 * (c + r) :
                                     1088 * h + 64 * (c + r) + 64]
                            mm(op[64 * hh : 64 * hh + 64, 128 * t : 128 * t + 128],
                               vsl,
                               pt[:, b0 + 256 * hh + 128 * r :
                                  b0 + 256 * hh + 128 * r + 128],
                               start=(r == 0), stop=(r == 1))
                # stacked reciprocal [128,512] (partition-shifted halves) so
                # normalization is a single multiply
                rdn = pers.tile([128, 512], f16, name=f"rdn{c}", tag="rdn",
                                bufs=3)
                nc.vector.reciprocal(out=rdn[0:64, :], in_=dn[:, 0:512])
                nc.vector.reciprocal(out=rdn[64:128, :], in_=dn[:, 512:1024])
                ot = pers.tile([128, 512], bf16, name=f"ot{c}", tag="ot", bufs=4)
                nc.vector.tensor_mul(out=ot[:], in0=op[:], in1=rdn[:])
                return c, ot

            def emit_outproj(c, ot):
                yp = ps.tile([128, 512], f32, name=f"yp{c}", tag="yp", bufs=1)
                # bias via K=1 ones-matmul (skipped when bout is all zero)
                if with_bout:
                    mm(yp[:], onecol[:], boutr[:], start=True, stop=False)
                for kt in range(4):
                    mm(yp[:], ot[:, 128 * kt : 128 * kt + 128],
                       wo[:, 512 * kt : 512 * kt + 512],
                       start=(kt == 0 and not with_bout), stop=(kt == 3))
                ysb = pers.tile([128, 512], bf16, name=f"ysb{c}", tag="ysb",
                                bufs=3)
                nc.scalar.activation(out=ysb[:], in_=yp[:],
                     func=mybir.ActivationFunctionType.Copy,
                     scale=0.0625)
                nc.sync.dma_start(out=y_d[128 * c : 128 * c + 128, :], in_=ysb[:])

            # chunk c reads keys up to 128c+191, i.e. into l-block c//4 + 1,
            # so attention lags one chunk behind the projections. The chunk
            # stages are software-pipelined (scores(c) | pv+norm(c-1) |
            # outproj(c-2)) so no engine head-of-line blocks on another.
            chunk_ranges = [range(0, 3), range(3, 7), range(7, 11), range(11, 16)]
            pend_sc = None
            pend_oj = None
            for lc in range(4):
                for t in range(4):
                    emit_qk_proj(t, lc, is_k=0)
                for t in range(4):
                    emit_qk_proj(t, lc, is_k=1)
                for j in vtiles[lc]:
                    emit_v_proj(j)
                for c in chunk_ranges[lc]:
                    sc = emit_scores(c)
                    if pend_sc is not None:
                        if pend_oj is not None:
                            emit_outproj(*pend_oj)
                        pend_oj = emit_pvdn(pend_sc)
                    pend_sc = sc
            if pend_oj is not None:
                emit_outproj(*pend_oj)
            pend_oj = emit_pvdn(pend_sc)
            emit_outproj(*pend_oj)

    nc.compile()
    return nc


def get_nc(with_bout=False):
    key = ("nc", bool(with_bout))
    if key not in _NC_CACHE:
        _NC_CACHE[key] = _build_nc(with_bout=with_bout)
    return _NC_CACHE[key]


def make_core_inputs(x, Wqkv, Wout, bout):
    """Host-side shard + layout prep (cheap numpy transposes/casts)."""
    e4, e5, bf = (ml_dtypes.float8_e4m3, ml_dtypes.float8_e5m2,
                  ml_dtypes.bfloat16)
    x = np.asarray(x, dtype=np.float32)
    Wqkv = np.asarray(Wqkv, dtype=np.float32)
    Wout = np.asarray(Wout, dtype=np.float32)
    # ysb applies a 1/16 scale (V path carries 16x), so bias is 16x here
    boutr = np.ascontiguousarray(
        (np.asarray(bout, dtype=np.float32) * WS).reshape(1, 512).astype(bf)
    )

    # wqk8 [128, 4096]: col = 512t + 256jj + 128ii + m ; 16x scaled
    QK = (Wqkv[0:1024] * WS).astype(e4)  # [c, d]
    wqk8 = np.ascontiguousarray(
        QK.reshape(8, 128, 2, 2, 128).transpose(4, 0, 2, 3, 1).reshape(128, 4096)
    )
    # wv8 [128, 4096]: [wv8 | dwv8], col-within = 1024jj + 512ii + n;
    # 16x scaled with fp8 residual compensation
    WVs = Wqkv[1024:1536] * WS  # [n, d]
    WV8 = WVs.astype(e4).astype(np.float32)
    DWV8 = (WVs - WV8).astype(e4)

    def vlay(a):
        # [n, d] -> [p, jj, ii, n] with d = 256jj + 128ii + p
        return a.reshape(512, 2, 2, 128).transpose(3, 1, 2, 0).reshape(128, 2048)

    wv8 = np.ascontiguousarray(
        np.concatenate([vlay(WV8.astype(e4)), vlay(DWV8)], axis=1)
    )
    # wob [128, 2048]: col = 512kt + n
    WO = Wout.astype(bf)
    wob = np.ascontiguousarray(
        WO.reshape(512, 4, 128).transpose(2, 1, 0).reshape(128, 2048)
    )
    one8 = np.ones((128, 128), dtype=e4)
    # ce5 [128, 3328]: [I | Z] + 3 bias variants [ua ub ua ub | Z Z Z Z]
    pp, ff = np.mgrid[0:128, 0:128]
    ident = np.eye(128, dtype=np.float32)
    zero = np.zeros((128, 128), np.float32)
    u0 = np.where(pp >= ff, 0.0, NEG)
    u1 = np.where(pp <= ff, 0.0, NEG)
    u2 = np.where(pp < 64, NEG, u0)
    u3 = np.where(pp > 63, NEG, u1)
    z4 = np.concatenate([zero] * 4, axis=1)
    ce5 = np.concatenate(
        [ident, zero,
         u0, u1, u0, u1, z4,
         u2, u1, u2, u1, z4,
         u0, u3, u0, u3, z4], axis=1
    ).astype(e5)

    in_maps = []
    for b in range(B):
        xb_ = x[:, b, :]  # [L, D]
        # x8/dx8 [128, 8704]: col = 2176kb + (l+64), zero padded;
        # dx8 is the fp8 residual of x for the V-path compensation
        xpad = np.zeros((2176, 512), np.float32)
        xpad[64:2112] = xb_
        x8f = xpad.astype(e4)
        dxf = (xpad - x8f.astype(np.float32)).astype(e4)

        def xlay(a):
            return np.ascontiguousarray(
                a.reshape(2176, 4, 128).transpose(2, 1, 0).reshape(128, 8704)
            )

        in_maps.append(
            {
                "x8": xlay(x8f),
                "dx8": xlay(dxf),
                "wqk8": wqk8,
                "wv8": wv8,
                "wob": wob,
                "one8": one8,
                "ce5": ce5,
                "bout": boutr,
            }
        )
    return in_maps


def kernel(x, Wqkv, Wout, bout):
    from concourse.bass_utils import run_bass_kernel_spmd

    nc = get_nc(with_bout=bool(np.any(np.asarray(bout))))
    in_maps = make_core_inputs(x, Wqkv, Wout, bout)
    res = run_bass_kernel_spmd(nc, in_maps, core_ids=list(range(B)))
    out = np.empty((L, B, D), dtype=np.float32)
    for b in range(B):
        out[:, b, :] = res.results[b]["y"].astype(np.float32)
    return out
